# revision 21
# baseline (speedup 1.0000x reference)
"""Trainium2 Bass kernel for label-smoothed CE loss with over-confidence flipping.

Reference computation (B=2048, C=32000, pred in [0,1), label int64):
    probs = softmax(pred, axis=1)
    mask = probs > 0.5
    true_dist = where(mask, 0.1, 0) ; true_dist[r, label[r]] = 0.9
    pred_flipped = where(mask, 1-pred, pred)
    loss = mean_r sum_c -true_dist * log(pred_flipped)

Per-row this is
    loss_r = -0.9*log(pf[r, l_r]) - 0.1 * sum_{c in mask, c != l_r} log(1-pred[r,c])
and mask is empty for row r  <=>  max_c exp(pred[r,c]) <= S_r/2 where
S_r = sum_c exp(pred[r,c]).  For any pred in [0,1)^BxC the mask is empty with
a huge margin (it would need pred > ln(16000) ~ 9.68), so the loss is exactly
-0.9 * mean_r log(pred[r, label_r]).

The kernel shards the batch over 8 cores (256 rows each).  Each core streams
its full pred shard once from HBM (memory-bound) through an in-place ScalarE
Exp pass, producing the per-row softmax denominators S_r (accum_out), and
gathers pred[r, label_r] by indirect DMA, emitting ln of it.  The host then
verifies mask-emptiness exactly — exp(rowmax) <= 0.45*S_r (the 0.45 margin
absorbs f32 rounding of S) and S finite — and either returns
-0.9 * mean(ln g) or falls back to an exact recompute of the full formula
(unreachable for in-domain inputs).
"""

import numpy as np

import concourse.bass as bass
import concourse.tile as tile
from concourse import mybir
from concourse.bass_utils import run_bass_kernel_spmd

B = 2048
C = 32000
N_CORES = 8
ROWS = B // N_CORES  # 256 rows per core
P = 128              # partitions
RT = ROWS // P       # row-tiles per core
# Uneven column chunks: the last chunk of each row-tile is small so the
# final Exp after the DMA stream ends is short (tail latency).
CHUNKS = [4500] * 7 + [500]
NCH = len(CHUNKS)
assert sum(CHUNKS) == C
LABEL_SMOOTH = 0.1

_BUILT = None


def _build():
    f32 = mybir.dt.float32
    nc = bass.Bass()

    # pred is declared flat [ROWS*C, 1] so the same tensor can serve the
    # indirect gather (which requires offset 0 on the indirected AP).
    pred_in = nc.dram_tensor("pred", [ROWS * C, 1], f32, kind="ExternalInput")
    idx_in = nc.dram_tensor("idx", [RT, P, 1], mybir.dt.int32, kind="ExternalInput")
    # out[..., 0] = ln(pred[r, label_r]),  out[..., 1] = S_r
    out_t = nc.dram_tensor("out", [RT, P, 2], f32, kind="ExternalOutput")

    pred2d = pred_in.rearrange("(r c) o -> r (c o)", c=C)  # [ROWS, C]

    AF = mybir.ActivationFunctionType
    OP = mybir.AluOpType
    X = mybir.AxisListType.X

    with tile.TileContext(nc) as tc:
        # Every instruction on this pipeline supports a single explicit sync
        # wait, so the dataflow is kept strictly chain-shaped: each chunk is
        # loaded (HWDGE), consumed in place by the ScalarE Exp (whose
        # accum_out column is the only value kept), and the slot recycles.
        # Small transfers use SWDGE (gpsimd).
        with (
            tc.tile_pool(name="chunks", bufs=8) as chunk_pool,
            tc.tile_pool(name="small", bufs=2) as small,
        ):
            for t in range(RT):
                rows = slice(t * P, (t + 1) * P)
                sums = small.tile([P, NCH], f32)
                col = 0
                for ci, ch in enumerate(CHUNKS):
                    chunk = chunk_pool.tile([P, max(CHUNKS)], f32)
                    src = pred2d[rows, col:col + ch]
                    if t == 0 and ci == 0:
                        # Pool's SWDGE queue starts issuing several us before
                        # the SP preamble finishes — kick the HBM stream off
                        # early with the first chunk.
                        nc.gpsimd.dma_start(chunk[:, :ch], src)
                    else:
                        nc.sync.dma_start(chunk[:, :ch], src)
                    nc.scalar.activation(
                        chunk[:, :ch], chunk[:, :ch], AF.Exp,
                        accum_out=sums[:, ci:ci + 1],
                    )
                    col += ch

                S = small.tile([P, 1], f32)
                nc.vector.tensor_reduce(S[:], sums[:], axis=X, op=OP.add)

                # gather pred[r, label_r] and take its log; combine with S
                # into one ScalarE-written tile so a single DMA emits both.
                idxt = small.tile([P, 1], mybir.dt.int32)
                nc.gpsimd.dma_start(idxt[:], idx_in[t, :, :])
                g = small.tile([P, 1], f32)
                nc.gpsimd.indirect_dma_start(
                    out=g[:],
                    out_offset=None,
                    in_=pred_in[:, :],
                    in_offset=bass.IndirectOffsetOnAxis(ap=idxt[:, :1], axis=0),
                )
                combo = small.tile([P, 2], f32)
                nc.scalar.activation(combo[:, 0:1], g[:], AF.Ln)
                nc.scalar.copy(combo[:, 1:2], S[:])
                nc.gpsimd.dma_start(out_t[t, :, :], combo[:])

    _fix_sync_waits(nc)
    return nc


def _fix_sync_waits(nc):
    """Fit Tile's emitted synchronization to the 1-sync-wait ISA descriptors.

    * Recycled chunk-slot loads carry a WAR/WAW wait on ScalarE (the slot's
      only consumer) plus a WAW wait on the slot's previous DMA lane.  The
      lane wait is transitively implied by the ScalarE wait (the in-place Exp
      RAW-waited on exactly that DMA), but Tile's wait assignment is not
      transitively minimal — drop it.
    * An in-place Exp may carry, besides its RAW DMA wait, a same-engine WAW
      wait against the Exp that previously owned the slot; it is likewise
      transitively implied (this Exp's DMA WAR-waited on that one) — drop it.
    * The kernel-tail Drain waits on every used semaphore; most are
      transitively implied — every chunk-DMA lane and the gather/idx lanes
      are covered by the final Activation tick (each Exp/Ln waited on its
      DMA), and DVE's final tick is covered by the ScalarE copy of S.  Only
      the ExternalOutput DMA lanes are observed by nothing else.  Keep
      {Activation, output lanes}, and hoist all but one onto single-wait
      EventSemaphore ops on the same sequencer.
    """
    out_lanes = set()
    for b in nc.m.functions[0].blocks:
        for inst in b.instructions:
            if type(inst).__name__ == "InstDMACopy" and "@out" in inst.concise():
                si = inst.sync_info
                if si is not None:
                    for u in si.on_update or []:
                        out_lanes.add(u.ant_name)
    assert len(out_lanes) == RT, out_lanes

    for b in nc.m.functions[0].blocks:
        insts = list(b.instructions)
        new_insts = []
        changed = False
        for inst in insts:
            kind = type(inst).__name__
            si = inst.sync_info
            ws = list(si.on_wait or []) if si is not None else []
            if kind == "InstDMACopy" and len(ws) > 1 and "@chunk" in inst.concise():
                names = [w.ant_name or "" for w in ws]
                assert any(n.startswith("Activation") for n in names), (
                    f"unexpected chunk DMA waits {inst.name}: {inst.concise()[:200]}"
                )
                si.on_wait = [
                    w for w in ws
                    if not w.ant_name.startswith(("DMAHW", "DMASW"))
                ]
                assert len(si.on_wait) <= 1, inst.concise()[:200]
            elif (
                kind == "InstActivation"
                and len(ws) > 1
                and "@chunk" in inst.concise()
            ):
                si.on_wait = [
                    w for w in ws if not w.ant_name.startswith("Activation")
                ]
                assert len(si.on_wait) <= 1, inst.concise()[:200]
            elif kind == "InstDrain" and len(ws) > 1:
                keep = []
                for w in ws:
                    n = w.ant_name or ""
                    if n.startswith("Activation") or n in out_lanes:
                        keep.append(w)
                    else:
                        assert n.startswith(("DMAHW", "DMASW", "DVE")), n
                for j, w in enumerate(keep[:-1]):
                    ev = mybir.InstEventSemaphore(
                        name=f"{inst.name}-presync-{j}",
                        engine=inst.engine,
                        sync_info=mybir.SyncInfo(on_wait=[w], on_update=[]),
                    )
                    try:
                        nc.register_instruction(ev, overwrite=True)
                    except Exception:
                        pass
                    new_insts.append(ev)
                si.on_wait = [keep[-1]]
                changed = True
            new_insts.append(inst)
        if changed:
            b.instructions = new_insts


def _get_built():
    global _BUILT
    if _BUILT is None:
        _BUILT = _build()
    return _BUILT


def _host_fallback(pred, label):
    # Exact recompute (float64) of the reference for out-of-domain inputs
    # (only reachable when some softmax prob approaches/exceeds 0.5, which is
    # impossible for pred in [0,1) with C=32000).
    p = pred.astype(np.float64)
    mx = p.max(axis=1, keepdims=True)
    e = np.exp(p - mx)
    probs = e / e.sum(axis=1, keepdims=True)
    mask = probs > 0.5
    true_dist = np.where(mask, LABEL_SMOOTH, 0.0)
    rows = np.arange(p.shape[0])
    true_dist[rows, label] = 1.0 - LABEL_SMOOTH
    pf = np.where(mask, 1.0 - p, p)
    with np.errstate(divide="ignore", invalid="ignore"):
        per_row = np.sum(-true_dist * np.log(pf), axis=1)
    return np.array(per_row.mean(), dtype=np.float32)


def _run(pred, label, trace=False):
    pred = np.ascontiguousarray(np.asarray(pred), dtype=np.float32)
    label = np.asarray(label).astype(np.int64)
    assert pred.shape == (B, C) and label.shape == (B,)

    nc = _get_built()
    local_r = np.arange(ROWS, dtype=np.int64) * C
    in_maps = []
    for k in range(N_CORES):
        sl = slice(k * ROWS, (k + 1) * ROWS)
        idx = (local_r + label[sl]).astype(np.int32).reshape(RT, P, 1)
        in_maps.append({
            "pred": pred[sl].reshape(ROWS * C, 1),
            "idx": idx,
        })

    res = run_bass_kernel_spmd(
        nc, in_maps, core_ids=list(range(N_CORES)), trace=trace,
    )
    lng = np.concatenate([r["out"][:, :, 0].reshape(-1) for r in res.results])
    S = np.concatenate([r["out"][:, :, 1].reshape(-1) for r in res.results])

    # Exact mask-emptiness guard: mask empty for row r iff
    # exp(max_c pred) <= S_r/2.  The 0.45 factor absorbs f32 rounding of S;
    # for in-domain inputs exp(rowmax) < e while 0.45*S > 13000.
    rowmax = pred.max(axis=1).astype(np.float64)
    with np.errstate(over="ignore"):
        ok = np.isfinite(S).all() and bool(
            np.all(np.exp(rowmax) <= 0.45 * S.astype(np.float64))
        )
    if not ok:
        return _host_fallback(pred, label), res

    # IEEE faithfulness: the reference's sum includes 0*log(pred[r,c]) for
    # every non-label column, which is NaN wherever pred <= 0 (0*(-inf) and
    # 0*NaN are both NaN), while a nonpositive label entry gives
    # -0.9*log(g) = +inf (g == 0) or NaN (g < 0).  jax.random.uniform does
    # emit exact zeros (~2^-23 rate), so reproduce those semantics.
    if pred.min() <= 0.0:
        rows = np.arange(B)
        g = pred[rows, label].astype(np.float64)
        nonpos_cnt = (pred <= 0.0).sum(axis=1)
        nonpos_nonlabel = (nonpos_cnt - (g <= 0.0)) > 0
        with np.errstate(divide="ignore", invalid="ignore"):
            per_row = -(1.0 - LABEL_SMOOTH) * np.log(g)
        per_row[nonpos_nonlabel] = np.nan
        return np.array(per_row.mean(), dtype=np.float32), res

    loss = -(1.0 - LABEL_SMOOTH) * lng.mean(dtype=np.float64)
    return np.array(loss, dtype=np.float32), res


def kernel(pred, label):
    out, _ = _run(pred, label)
    return out


# revision 23
# speedup vs baseline: 1.0403x; 1.0403x over previous
"""Trainium2 Bass kernel for label-smoothed CE loss with over-confidence flipping.

Reference computation (B=2048, C=32000, pred in [0,1), label int64):
    probs = softmax(pred, axis=1)
    mask = probs > 0.5
    true_dist = where(mask, 0.1, 0) ; true_dist[r, label[r]] = 0.9
    pred_flipped = where(mask, 1-pred, pred)
    loss = mean_r sum_c -true_dist * log(pred_flipped)

Per-row this is
    loss_r = -0.9*log(pf[r, l_r]) - 0.1 * sum_{c in mask, c != l_r} log(1-pred[r,c])
and mask is empty for row r  <=>  max_c exp(pred[r,c]) <= S_r/2 where
S_r = sum_c exp(pred[r,c]).  For any pred in [0,1)^BxC the mask is empty with
a huge margin (it would need pred > ln(16000) ~ 9.68), so the loss is exactly
-0.9 * mean_r log(pred[r, label_r]).

The kernel shards the batch over 8 cores (256 rows each).  Each core streams
its full pred shard once from HBM (memory-bound) through an in-place ScalarE
Exp pass, producing the per-row softmax denominators S_r (accum_out), and
gathers pred[r, label_r] by indirect DMA, emitting ln of it.  The host then
verifies mask-emptiness exactly — exp(rowmax) <= 0.45*S_r (the 0.45 margin
absorbs f32 rounding of S) and S finite — and either returns
-0.9 * mean(ln g) or falls back to an exact recompute of the full formula
(unreachable for in-domain inputs).
"""

import numpy as np

import concourse.bass as bass
import concourse.tile as tile
from concourse import mybir
from concourse.bass_utils import run_bass_kernel_spmd

B = 2048
C = 32000
N_CORES = 8
ROWS = B // N_CORES  # 256 rows per core
P = 128              # partitions
RT = ROWS // P       # row-tiles per core
# Uneven column chunks: the last chunk of each row-tile is small so the
# final Exp after the DMA stream ends is short (tail latency).  Chunk size
# must keep Exp time + semaphore latency under the chunk's DMA time
# (~5.5us) or the slot-recycle feedback loop starves the DMA queue.
CHUNKS = [3937] * 8 + [504]
NCH = len(CHUNKS)
assert sum(CHUNKS) == C
LABEL_SMOOTH = 0.1

_BUILT = None


def _build():
    f32 = mybir.dt.float32
    nc = bass.Bass()

    # pred is declared flat [ROWS*C, 1] so the same tensor can serve the
    # indirect gather (which requires offset 0 on the indirected AP).
    pred_in = nc.dram_tensor("pred", [ROWS * C, 1], f32, kind="ExternalInput")
    idx_in = nc.dram_tensor("idx", [RT, P, 1], mybir.dt.int32, kind="ExternalInput")
    # out[..., 0] = ln(pred[r, label_r]),  out[..., 1] = S_r
    out_t = nc.dram_tensor("out", [RT, P, 2], f32, kind="ExternalOutput")

    pred2d = pred_in.rearrange("(r c) o -> r (c o)", c=C)  # [ROWS, C]

    AF = mybir.ActivationFunctionType
    OP = mybir.AluOpType
    X = mybir.AxisListType.X

    with tile.TileContext(nc) as tc:
        # Every instruction on this pipeline supports a single explicit sync
        # wait, so the dataflow is kept strictly chain-shaped: each chunk is
        # loaded (HWDGE), consumed in place by the ScalarE Exp (whose
        # accum_out column is the only value kept), and the slot recycles.
        # Small transfers use SWDGE (gpsimd).
        with (
            tc.tile_pool(name="chunks", bufs=8) as chunk_pool,
            tc.tile_pool(name="small", bufs=2) as small,
        ):
            for t in range(RT):
                rows = slice(t * P, (t + 1) * P)
                sums = small.tile([P, NCH], f32)
                col = 0
                for ci, ch in enumerate(CHUNKS):
                    chunk = chunk_pool.tile([P, max(CHUNKS)], f32)
                    nc.sync.dma_start(chunk[:, :ch], pred2d[rows, col:col + ch])
                    nc.scalar.activation(
                        chunk[:, :ch], chunk[:, :ch], AF.Exp,
                        accum_out=sums[:, ci:ci + 1],
                    )
                    col += ch

                S = small.tile([P, 1], f32)
                nc.vector.tensor_reduce(S[:], sums[:], axis=X, op=OP.add)

                # gather pred[r, label_r] and take its log; combine with S
                # into one ScalarE-written tile so a single DMA emits both.
                idxt = small.tile([P, 1], mybir.dt.int32)
                nc.gpsimd.dma_start(idxt[:], idx_in[t, :, :])
                g = small.tile([P, 1], f32)
                nc.gpsimd.indirect_dma_start(
                    out=g[:],
                    out_offset=None,
                    in_=pred_in[:, :],
                    in_offset=bass.IndirectOffsetOnAxis(ap=idxt[:, :1], axis=0),
                )
                combo = small.tile([P, 2], f32)
                nc.scalar.activation(combo[:, 0:1], g[:], AF.Ln)
                nc.scalar.copy(combo[:, 1:2], S[:])
                nc.gpsimd.dma_start(out_t[t, :, :], combo[:])

    _fix_sync_waits(nc)
    return nc


def _fix_sync_waits(nc):
    """Fit Tile's emitted synchronization to the 1-sync-wait ISA descriptors.

    * Recycled chunk-slot loads carry a WAR/WAW wait on ScalarE (the slot's
      only consumer) plus a WAW wait on the slot's previous DMA lane.  The
      lane wait is transitively implied by the ScalarE wait (the in-place Exp
      RAW-waited on exactly that DMA), but Tile's wait assignment is not
      transitively minimal — drop it.
    * An in-place Exp may carry, besides its RAW DMA wait, a same-engine WAW
      wait against the Exp that previously owned the slot; it is likewise
      transitively implied (this Exp's DMA WAR-waited on that one) — drop it.
    * The kernel-tail Drain waits on every used semaphore; most are
      transitively implied — every chunk-DMA lane and the gather/idx lanes
      are covered by the final Activation tick (each Exp/Ln waited on its
      DMA), and DVE's final tick is covered by the ScalarE copy of S.  Only
      the ExternalOutput DMA lanes are observed by nothing else.  Keep
      {Activation, output lanes}, and hoist all but one onto single-wait
      EventSemaphore ops on the same sequencer.
    """
    out_lanes = set()
    for b in nc.m.functions[0].blocks:
        for inst in b.instructions:
            if type(inst).__name__ == "InstDMACopy" and "@out" in inst.concise():
                si = inst.sync_info
                if si is not None:
                    for u in si.on_update or []:
                        out_lanes.add(u.ant_name)
    assert len(out_lanes) == RT, out_lanes

    for b in nc.m.functions[0].blocks:
        insts = list(b.instructions)
        new_insts = []
        changed = False
        for inst in insts:
            kind = type(inst).__name__
            si = inst.sync_info
            ws = list(si.on_wait or []) if si is not None else []
            if kind == "InstDMACopy" and len(ws) > 1 and "@chunk" in inst.concise():
                names = [w.ant_name or "" for w in ws]
                assert any(n.startswith("Activation") for n in names), (
                    f"unexpected chunk DMA waits {inst.name}: {inst.concise()[:200]}"
                )
                si.on_wait = [
                    w for w in ws
                    if not w.ant_name.startswith(("DMAHW", "DMASW"))
                ]
                assert len(si.on_wait) <= 1, inst.concise()[:200]
            elif (
                kind == "InstActivation"
                and len(ws) > 1
                and "@chunk" in inst.concise()
            ):
                si.on_wait = [
                    w for w in ws if not w.ant_name.startswith("Activation")
                ]
                assert len(si.on_wait) <= 1, inst.concise()[:200]
            elif kind == "InstDrain" and len(ws) > 1:
                keep = []
                for w in ws:
                    n = w.ant_name or ""
                    if n.startswith("Activation") or n in out_lanes:
                        keep.append(w)
                    else:
                        assert n.startswith(("DMAHW", "DMASW", "DVE")), n
                for j, w in enumerate(keep[:-1]):
                    ev = mybir.InstEventSemaphore(
                        name=f"{inst.name}-presync-{j}",
                        engine=inst.engine,
                        sync_info=mybir.SyncInfo(on_wait=[w], on_update=[]),
                    )
                    try:
                        nc.register_instruction(ev, overwrite=True)
                    except Exception:
                        pass
                    new_insts.append(ev)
                si.on_wait = [keep[-1]]
                changed = True
            new_insts.append(inst)
        if changed:
            b.instructions = new_insts


def _get_built():
    global _BUILT
    if _BUILT is None:
        _BUILT = _build()
    return _BUILT


def _host_fallback(pred, label):
    # Exact recompute (float64) of the reference for out-of-domain inputs
    # (only reachable when some softmax prob approaches/exceeds 0.5, which is
    # impossible for pred in [0,1) with C=32000).
    p = pred.astype(np.float64)
    mx = p.max(axis=1, keepdims=True)
    e = np.exp(p - mx)
    probs = e / e.sum(axis=1, keepdims=True)
    mask = probs > 0.5
    true_dist = np.where(mask, LABEL_SMOOTH, 0.0)
    rows = np.arange(p.shape[0])
    true_dist[rows, label] = 1.0 - LABEL_SMOOTH
    pf = np.where(mask, 1.0 - p, p)
    with np.errstate(divide="ignore", invalid="ignore"):
        per_row = np.sum(-true_dist * np.log(pf), axis=1)
    return np.array(per_row.mean(), dtype=np.float32)


def _run(pred, label, trace=False):
    pred = np.ascontiguousarray(np.asarray(pred), dtype=np.float32)
    label = np.asarray(label).astype(np.int64)
    assert pred.shape == (B, C) and label.shape == (B,)

    nc = _get_built()
    local_r = np.arange(ROWS, dtype=np.int64) * C
    in_maps = []
    for k in range(N_CORES):
        sl = slice(k * ROWS, (k + 1) * ROWS)
        idx = (local_r + label[sl]).astype(np.int32).reshape(RT, P, 1)
        in_maps.append({
            "pred": pred[sl].reshape(ROWS * C, 1),
            "idx": idx,
        })

    res = run_bass_kernel_spmd(
        nc, in_maps, core_ids=list(range(N_CORES)), trace=trace,
    )
    lng = np.concatenate([r["out"][:, :, 0].reshape(-1) for r in res.results])
    S = np.concatenate([r["out"][:, :, 1].reshape(-1) for r in res.results])

    # Exact mask-emptiness guard: mask empty for row r iff
    # exp(max_c pred) <= S_r/2.  The 0.45 factor absorbs f32 rounding of S;
    # for in-domain inputs exp(rowmax) < e while 0.45*S > 13000.
    rowmax = pred.max(axis=1).astype(np.float64)
    with np.errstate(over="ignore"):
        ok = np.isfinite(S).all() and bool(
            np.all(np.exp(rowmax) <= 0.45 * S.astype(np.float64))
        )
    if not ok:
        return _host_fallback(pred, label), res

    # IEEE faithfulness: the reference's sum includes 0*log(pred[r,c]) for
    # every non-label column, which is NaN wherever pred <= 0 (0*(-inf) and
    # 0*NaN are both NaN), while a nonpositive label entry gives
    # -0.9*log(g) = +inf (g == 0) or NaN (g < 0).  jax.random.uniform does
    # emit exact zeros (~2^-23 rate), so reproduce those semantics.
    if pred.min() <= 0.0:
        rows = np.arange(B)
        g = pred[rows, label].astype(np.float64)
        nonpos_cnt = (pred <= 0.0).sum(axis=1)
        nonpos_nonlabel = (nonpos_cnt - (g <= 0.0)) > 0
        with np.errstate(divide="ignore", invalid="ignore"):
            per_row = -(1.0 - LABEL_SMOOTH) * np.log(g)
        per_row[nonpos_nonlabel] = np.nan
        return np.array(per_row.mean(), dtype=np.float32), res

    loss = -(1.0 - LABEL_SMOOTH) * lng.mean(dtype=np.float64)
    return np.array(loss, dtype=np.float32), res


def kernel(pred, label):
    out, _ = _run(pred, label)
    return out


# revision 24
# speedup vs baseline: 1.1775x; 1.1319x over previous
"""Trainium2 Bass kernel for label-smoothed CE loss with over-confidence flipping.

Reference computation (B=2048, C=32000, pred in [0,1), label int64):
    probs = softmax(pred, axis=1)
    mask = probs > 0.5
    true_dist = where(mask, 0.1, 0) ; true_dist[r, label[r]] = 0.9
    pred_flipped = where(mask, 1-pred, pred)
    loss = mean_r sum_c -true_dist * log(pred_flipped)

Per-row this is
    loss_r = -0.9*log(pf[r, l_r]) - 0.1 * sum_{c in mask, c != l_r} log(1-pred[r,c])
and mask is empty for row r  <=>  max_c exp(pred[r,c]) <= S_r/2 where
S_r = sum_c exp(pred[r,c]).  For any pred in [0,1)^BxC the mask is empty with
a huge margin (it would need pred > ln(16000) ~ 9.68), so the loss is exactly
-0.9 * mean_r log(pred[r, label_r]).

The kernel shards the batch over 8 cores (256 rows each).  Each core streams
its full pred shard once from HBM (memory-bound) through an in-place ScalarE
Exp pass, producing the per-row softmax denominators S_r (accum_out), and
gathers pred[r, label_r] by indirect DMA, emitting ln of it.  The host then
verifies mask-emptiness exactly — exp(rowmax) <= 0.45*S_r (the 0.45 margin
absorbs f32 rounding of S) and S finite — and either returns
-0.9 * mean(ln g) or falls back to an exact recompute of the full formula
(unreachable for in-domain inputs).
"""

import numpy as np

import concourse.bass as bass
import concourse.tile as tile
from concourse import mybir
from concourse.bass_utils import run_bass_kernel_spmd

B = 2048
C = 32000
N_CORES = 8
ROWS = B // N_CORES  # 256 rows per core
P = 128              # partitions
RT = ROWS // P       # row-tiles per core
# Even 4000-column chunks measured best (368 GB/s sustained): Exp time +
# semaphore latency (~4.7us) must stay under the chunk's DMA time (~5.6us)
# with ~1us of margin, or the slot-recycle feedback loop starves the DMA
# queue (uneven/larger chunk splits measured 332-344 GB/s).
CHUNKS = [4000] * 8
NCH = len(CHUNKS)
assert sum(CHUNKS) == C
LABEL_SMOOTH = 0.1

_BUILT = None


def _build():
    f32 = mybir.dt.float32
    nc = bass.Bass()

    # pred is declared flat [ROWS*C, 1] so the same tensor can serve the
    # indirect gather (which requires offset 0 on the indirected AP).
    pred_in = nc.dram_tensor("pred", [ROWS * C, 1], f32, kind="ExternalInput")
    idx_in = nc.dram_tensor("idx", [RT, P, 1], mybir.dt.int32, kind="ExternalInput")
    # out[..., 0] = ln(pred[r, label_r]),  out[..., 1] = S_r
    out_t = nc.dram_tensor("out", [RT, P, 2], f32, kind="ExternalOutput")

    pred2d = pred_in.rearrange("(r c) o -> r (c o)", c=C)  # [ROWS, C]

    AF = mybir.ActivationFunctionType
    OP = mybir.AluOpType
    X = mybir.AxisListType.X

    with tile.TileContext(nc) as tc:
        # Every instruction on this pipeline supports a single explicit sync
        # wait, so the dataflow is kept strictly chain-shaped: each chunk is
        # loaded (HWDGE), consumed in place by the ScalarE Exp (whose
        # accum_out column is the only value kept), and the slot recycles.
        # Small transfers use SWDGE (gpsimd).
        with (
            tc.tile_pool(name="chunks", bufs=8) as chunk_pool,
            tc.tile_pool(name="small", bufs=2) as small,
        ):
            for t in range(RT):
                rows = slice(t * P, (t + 1) * P)
                sums = small.tile([P, NCH], f32)
                col = 0
                for ci, ch in enumerate(CHUNKS):
                    chunk = chunk_pool.tile([P, max(CHUNKS)], f32)
                    nc.sync.dma_start(chunk[:, :ch], pred2d[rows, col:col + ch])
                    nc.scalar.activation(
                        chunk[:, :ch], chunk[:, :ch], AF.Exp,
                        accum_out=sums[:, ci:ci + 1],
                    )
                    col += ch

                S = small.tile([P, 1], f32)
                nc.vector.tensor_reduce(S[:], sums[:], axis=X, op=OP.add)

                # gather pred[r, label_r] and take its log; combine with S
                # into one ScalarE-written tile so a single DMA emits both.
                idxt = small.tile([P, 1], mybir.dt.int32)
                nc.gpsimd.dma_start(idxt[:], idx_in[t, :, :])
                g = small.tile([P, 1], f32)
                nc.gpsimd.indirect_dma_start(
                    out=g[:],
                    out_offset=None,
                    in_=pred_in[:, :],
                    in_offset=bass.IndirectOffsetOnAxis(ap=idxt[:, :1], axis=0),
                )
                combo = small.tile([P, 2], f32)
                nc.scalar.activation(combo[:, 0:1], g[:], AF.Ln)
                nc.scalar.copy(combo[:, 1:2], S[:])
                nc.gpsimd.dma_start(out_t[t, :, :], combo[:])

    _fix_sync_waits(nc)
    return nc


def _fix_sync_waits(nc):
    """Fit Tile's emitted synchronization to the 1-sync-wait ISA descriptors.

    * Recycled chunk-slot loads carry a WAR/WAW wait on ScalarE (the slot's
      only consumer) plus a WAW wait on the slot's previous DMA lane.  The
      lane wait is transitively implied by the ScalarE wait (the in-place Exp
      RAW-waited on exactly that DMA), but Tile's wait assignment is not
      transitively minimal — drop it.
    * An in-place Exp may carry, besides its RAW DMA wait, a same-engine WAW
      wait against the Exp that previously owned the slot; it is likewise
      transitively implied (this Exp's DMA WAR-waited on that one) — drop it.
    * The kernel-tail Drain waits on every used semaphore; most are
      transitively implied — every chunk-DMA lane and the gather/idx lanes
      are covered by the final Activation tick (each Exp/Ln waited on its
      DMA), and DVE's final tick is covered by the ScalarE copy of S.  Only
      the ExternalOutput DMA lanes are observed by nothing else.  Keep
      {Activation, output lanes}, and hoist all but one onto single-wait
      EventSemaphore ops on the same sequencer.
    """
    out_lanes = set()
    for b in nc.m.functions[0].blocks:
        for inst in b.instructions:
            if type(inst).__name__ == "InstDMACopy" and "@out" in inst.concise():
                si = inst.sync_info
                if si is not None:
                    for u in si.on_update or []:
                        out_lanes.add(u.ant_name)
    assert len(out_lanes) == RT, out_lanes

    for b in nc.m.functions[0].blocks:
        insts = list(b.instructions)
        new_insts = []
        changed = False
        for inst in insts:
            kind = type(inst).__name__
            si = inst.sync_info
            ws = list(si.on_wait or []) if si is not None else []
            if kind == "InstDMACopy" and len(ws) > 1 and "@chunk" in inst.concise():
                names = [w.ant_name or "" for w in ws]
                assert any(n.startswith("Activation") for n in names), (
                    f"unexpected chunk DMA waits {inst.name}: {inst.concise()[:200]}"
                )
                si.on_wait = [
                    w for w in ws
                    if not w.ant_name.startswith(("DMAHW", "DMASW"))
                ]
                assert len(si.on_wait) <= 1, inst.concise()[:200]
            elif (
                kind == "InstActivation"
                and len(ws) > 1
                and "@chunk" in inst.concise()
            ):
                si.on_wait = [
                    w for w in ws if not w.ant_name.startswith("Activation")
                ]
                assert len(si.on_wait) <= 1, inst.concise()[:200]
            elif kind == "InstDrain" and len(ws) > 1:
                keep = []
                for w in ws:
                    n = w.ant_name or ""
                    if n.startswith("Activation") or n in out_lanes:
                        keep.append(w)
                    else:
                        assert n.startswith(("DMAHW", "DMASW", "DVE")), n
                for j, w in enumerate(keep[:-1]):
                    ev = mybir.InstEventSemaphore(
                        name=f"{inst.name}-presync-{j}",
                        engine=inst.engine,
                        sync_info=mybir.SyncInfo(on_wait=[w], on_update=[]),
                    )
                    try:
                        nc.register_instruction(ev, overwrite=True)
                    except Exception:
                        pass
                    new_insts.append(ev)
                si.on_wait = [keep[-1]]
                changed = True
            new_insts.append(inst)
        if changed:
            b.instructions = new_insts


def _get_built():
    global _BUILT
    if _BUILT is None:
        _BUILT = _build()
    return _BUILT


def _host_fallback(pred, label):
    # Exact recompute (float64) of the reference for out-of-domain inputs
    # (only reachable when some softmax prob approaches/exceeds 0.5, which is
    # impossible for pred in [0,1) with C=32000).
    p = pred.astype(np.float64)
    mx = p.max(axis=1, keepdims=True)
    e = np.exp(p - mx)
    probs = e / e.sum(axis=1, keepdims=True)
    mask = probs > 0.5
    true_dist = np.where(mask, LABEL_SMOOTH, 0.0)
    rows = np.arange(p.shape[0])
    true_dist[rows, label] = 1.0 - LABEL_SMOOTH
    pf = np.where(mask, 1.0 - p, p)
    with np.errstate(divide="ignore", invalid="ignore"):
        per_row = np.sum(-true_dist * np.log(pf), axis=1)
    return np.array(per_row.mean(), dtype=np.float32)


def _run(pred, label, trace=False):
    pred = np.ascontiguousarray(np.asarray(pred), dtype=np.float32)
    label = np.asarray(label).astype(np.int64)
    assert pred.shape == (B, C) and label.shape == (B,)

    nc = _get_built()
    local_r = np.arange(ROWS, dtype=np.int64) * C
    in_maps = []
    for k in range(N_CORES):
        sl = slice(k * ROWS, (k + 1) * ROWS)
        idx = (local_r + label[sl]).astype(np.int32).reshape(RT, P, 1)
        in_maps.append({
            "pred": pred[sl].reshape(ROWS * C, 1),
            "idx": idx,
        })

    res = run_bass_kernel_spmd(
        nc, in_maps, core_ids=list(range(N_CORES)), trace=trace,
    )
    lng = np.concatenate([r["out"][:, :, 0].reshape(-1) for r in res.results])
    S = np.concatenate([r["out"][:, :, 1].reshape(-1) for r in res.results])

    # Exact mask-emptiness guard: mask empty for row r iff
    # exp(max_c pred) <= S_r/2.  The 0.45 factor absorbs f32 rounding of S;
    # for in-domain inputs exp(rowmax) < e while 0.45*S > 13000.
    rowmax = pred.max(axis=1).astype(np.float64)
    with np.errstate(over="ignore"):
        ok = np.isfinite(S).all() and bool(
            np.all(np.exp(rowmax) <= 0.45 * S.astype(np.float64))
        )
    if not ok:
        return _host_fallback(pred, label), res

    # IEEE faithfulness: the reference's sum includes 0*log(pred[r,c]) for
    # every non-label column, which is NaN wherever pred <= 0 (0*(-inf) and
    # 0*NaN are both NaN), while a nonpositive label entry gives
    # -0.9*log(g) = +inf (g == 0) or NaN (g < 0).  jax.random.uniform does
    # emit exact zeros (~2^-23 rate), so reproduce those semantics.
    if pred.min() <= 0.0:
        rows = np.arange(B)
        g = pred[rows, label].astype(np.float64)
        nonpos_cnt = (pred <= 0.0).sum(axis=1)
        nonpos_nonlabel = (nonpos_cnt - (g <= 0.0)) > 0
        with np.errstate(divide="ignore", invalid="ignore"):
            per_row = -(1.0 - LABEL_SMOOTH) * np.log(g)
        per_row[nonpos_nonlabel] = np.nan
        return np.array(per_row.mean(), dtype=np.float32), res

    loss = -(1.0 - LABEL_SMOOTH) * lng.mean(dtype=np.float64)
    return np.array(loss, dtype=np.float32), res


def kernel(pred, label):
    out, _ = _run(pred, label)
    return out
